# revision 25
# baseline (speedup 1.0000x reference)
"""Trainium2 Bass kernel for nn_AutoRegressive_12128987644588.

6-layer post-norm transformer decoder (self-attn w/ prefix-causal mask,
cross-attn to packed embeddings, FFN), B=4, seq 865 (pad 896), D=1024,
16 heads x 64, FF=4096, final proj to 1024.

Sharding: 8 cores = 4 batches x 2 sequence halves of 448 tokens.
Per layer the two cores of a batch AllGather their x^T halves (the only
collective); K/V projections are computed over the full sequence on both
cores (duplicate compute, no other comm). Activations live transposed
[feature, token] in SBUF so every GEMM is natural.

v2 performance structure:
 - Attention processes heads in pairs (duos): the two S^T matmuls share
   the PE via row groups, their exps fuse into ONE strided ScalarE op
   over a 2-bank PSUM tile, halving ACT instruction count.
 - SA causal key tiles 5,6 are computed only for query cols 192:448
   (the only queries that can see keys >= 640); masks are per-core data.
 - Cross-attention K/V GEMMs are split into "units" drained into the
   ScalarE-bound attention windows and the AllGather gap, keeping the
   PE dense (and the HAM clock warm): K-units of layer l+1 fill layer
   l's CA window; V-units of layer l fill its own AG gap + SA window.
 - LN stats (col-sum matmuls + squares) are emitted inside the
   producing GEMM's evictions; LN tail scale/var ops run on DVE.
 - Weight pools are split (attention-band vs FFN) and next layer's Q
   weights are prefetched before the LN3 tail.
"""
import numpy as np

import concourse.bass as bass
import concourse.mybir as mybir
import concourse.tile as tile
from concourse import bacc, bass_utils

F32 = mybir.dt.float32
F32R = mybir.dt.float32r
BF16 = mybir.dt.float16  # fp16: FWL-eligible, 10-bit mantissa

B, D, H, HD, FF, L = 4, 1024, 16, 64, 4096, 6
TT, TA, ENR = 128, 512, 225
SEQ = TT + TA + ENR            # 865
TPAD = 896                     # 7 * 128
TH = 448                       # per-core half (padded)
PREFIX = TT + TA               # 640 = 5 * 128
NKT = TPAD // 128              # 7 key tiles
ND = D // 128                  # 8 feature tiles
VOCAB = 1024
EPS = 1e-5
NEG = -1e9
RQ0 = 192                      # causal tiles restricted to q cols RQ0:TH


# ---------------------------------------------------------------- host side

def sinusoidal_pe(T, d):
    pos = np.arange(T, dtype=np.float32)[:, None]
    div = np.exp(np.arange(0, d, 2, dtype=np.float32) * (-np.log(10000.0) / d))
    pe = np.zeros((T, d), dtype=np.float32)
    pe[:, 0::2] = np.sin(pos * div)
    pe[:, 1::2] = np.cos(pos * div)
    return pe


def host_embed(text, audio, enrolled_audio, text_len, audio_len,
               text_emb, audio_emb):
    """Replicates reference embed+pack. Returns [B, TPAD, D] f32 (pad zeros)."""
    te = text_emb[text] + sinusoidal_pe(TT, D)[None]        # [B,TT,D]
    ae = audio_emb[audio] + sinusoidal_pe(TA, D)[None]      # [B,TA,D]
    ee = audio_emb[enrolled_audio] + sinusoidal_pe(ENR, D)[None]
    out = np.zeros((B, TPAD, D), dtype=np.float32)
    for b in range(B):
        tl, al = int(text_len[b]), int(audio_len[b])
        out[b, :tl] = te[b, :tl]
        out[b, tl:tl + al] = ae[b, :al]
        out[b, tl + al:tl + al + ENR] = ee[b]
    return out


def host_masks(half):
    """Additive masks for SA key tiles 5,6, restricted to q cols RQ0:TH.
    Returns [256, TH - RQ0]: rows 0:128 = tile5 keys, 128:256 = tile6."""
    k = np.arange(PREFIX, PREFIX + 256)[:, None]            # 640..895
    q = half * TH + np.arange(RQ0, TH)[None, :]
    blocked = (k > q) | (k >= SEQ)
    return np.where(blocked, NEG, 0.0).astype(np.float32)


def host_kvalid6():
    k = PREFIX + 128 + np.arange(128)                       # 768..895
    return np.where(k < SEQ, 0.0, NEG).astype(np.float32)[:, None]


# ---------------------------------------------------------------- builder

def build_kernel(n_layers=L, skip_bv=False):
    nc = bacc.Bacc("TRN2", target_bir_lowering=False, debug=False,
                   num_devices=8)

    def din(name, shape, dt=F32R):
        return nc.dram_tensor(name, shape, dt, kind="ExternalInput")

    xT0_d = din("xT0", [D, TH])
    memT_d = din("memT", [D, TPAD], BF16)
    maskT_d = din("maskT", [256, TH - RQ0], F32)
    kval6_d = din("kval6", [128, 1], F32)
    ones_col_d = din("ones_col", [128, 1])
    ones_r128_d = din("ones_r128", [1, 128])
    vones_d = din("vones", [128, H], BF16)
    k2sel_d = din("k2sel", [2, 128])
    neg_r448_d = din("neg_r448", [1, TH])

    sa_inT_d = din("sa_inT", [L, D, 3 * D], BF16)
    sa_outT_d = din("sa_outT", [L, D, D], BF16)
    ca_inT_d = din("ca_inT", [L, D, 3 * D], BF16)
    ca_outT_d = din("ca_outT", [L, D, D], BF16)
    ff1T_d = din("ff1T", [L, D, FF], BF16)
    ff2T_d = din("ff2T", [L, FF, D], BF16)
    outT_d = din("outT", [D, VOCAB], BF16)

    sa_inb_d = din("sa_inb", [L, 3 * D], F32)
    sa_outb_d = din("sa_outb", [L, D], F32)
    ca_inb_d = din("ca_inb", [L, 3 * D], F32)
    ca_outb_d = din("ca_outb", [L, D], F32)
    ff1b_d = din("ff1b", [L, FF], F32)
    ff2b_d = din("ff2b", [L, D], F32)
    outb_d = din("outb", [VOCAB], F32)
    lnw_d = [din(f"ln{i}w", [L, D], F32) for i in (1, 2, 3)]
    lnb_d = [din(f"ln{i}b", [L, D], F32) for i in (1, 2, 3)]

    yT_d = nc.dram_tensor("yT", [VOCAB, TH], F32, kind="ExternalOutput")

    uid = [0]

    def nm(p):
        uid[0] += 1
        return f"{p}_{uid[0]}"

    AOP = mybir.AluOpType
    AF = mybir.ActivationFunctionType

    with tile.TileContext(nc) as tc:
        with (
            nc.allow_low_precision(reason="f32r compute; tol 2e-2"),
            tc.tile_pool(name="const", bufs=1) as constp,
            tc.tile_pool(name="xpool", bufs=8) as xpool,
            tc.tile_pool(name="memp", bufs=8) as memp,
            tc.tile_pool(name="cakv", bufs=2) as cakvp,
            tc.tile_pool(name="tmpp", bufs=4) as tmpp,
            tc.tile_pool(name="rows", bufs=2) as rowp,
            tc.tile_pool(name="statp", bufs=2) as statp,
            tc.tile_pool(name="biasp", bufs=12) as biasp,
            tc.tile_pool(name="wA", bufs=14) as wA,
            tc.tile_pool(name="wB", bufs=8) as wB,
            tc.tile_pool(name="wU", bufs=10) as wU,
            tc.tile_pool(name="upsp", bufs=1, space="PSUM") as upsp,
            tc.tile_pool(name="dram", bufs=2, space="DRAM") as dramp,
        ):
            # ---- constants
            ones_col = constp.tile([128, 1], F32R, name="ones_col")
            ones_r128 = constp.tile([1, 128], F32R, name="ones_r128")
            vones = constp.tile([128, H], BF16, name="vones")
            kval6 = constp.tile([128, 1], F32, name="kval6")
            mask5 = constp.tile([128, TH - RQ0], F32, name="mask5")
            mask6 = constp.tile([128, TH - RQ0], F32, name="mask6")
            k2sel_a = constp.tile([1, 128], F32R, name="k2sel_a")
            k2sel_b = constp.tile([1, 128], F32R, name="k2sel_b")
            lnrhs = constp.tile([2, TH], F32R, name="lnrhs")
            nc.sync.dma_start(out=ones_col[:], in_=ones_col_d.ap())
            nc.sync.dma_start(out=ones_r128[:], in_=ones_r128_d.ap())
            nc.sync.dma_start(out=vones[:], in_=vones_d.ap())
            nc.sync.dma_start(out=kval6[:], in_=kval6_d.ap())
            nc.sync.dma_start(out=mask5[:], in_=maskT_d.ap()[0:128, :])
            nc.sync.dma_start(out=mask6[:], in_=maskT_d.ap()[128:256, :])
            nc.sync.dma_start(out=k2sel_a[:], in_=k2sel_d.ap()[0:1, :])
            nc.sync.dma_start(out=k2sel_b[:], in_=k2sel_d.ap()[1:2, :])
            nc.sync.dma_start(out=lnrhs[1:2, :], in_=neg_r448_d.ap())
            eps_tile = constp.tile([1, 1], F32, name="eps_tile")
            nc.vector.memset(eps_tile[:], EPS)

            # ---- persistent memory tiles (cross-attn source, static)
            memt = [memp.tile([128, TPAD], BF16, name=nm("memt"), tag="mem",
                              bufs=8) for _ in range(ND)]
            for t in range(ND):
                nc.sync.dma_start(
                    out=memt[t][:],
                    in_=memT_d.ap()[t * 128:(t + 1) * 128, :])

            # ---- x tiles: fixed, updated in place through the whole net
            x_cur = []
            xb16 = []
            for t in range(ND):
                xt = xpool.tile([128, TH], F32R, name=nm("x"), tag="x")
                nc.sync.dma_start(out=xt[:],
                                  in_=xT0_d.ap()[t * 128:(t + 1) * 128, :])
                x_cur.append(xt)
                xb = xpool.tile([128, TH], BF16, name=nm("xb"), tag="xb")
                nc.vector.tensor_copy(xb[:], xt[:])
                xb16.append(xb)

            # ---------------------------------------------------- helpers
            def load_bias_col(src_1d_ap, n, name):
                t = biasp.tile([128, n], F32, name=nm(name), tag="bcol")
                nc.sync.dma_start(
                    out=t[:], in_=src_1d_ap.rearrange("(c p) -> p c", p=128))
                return t

            def load_row(src_1d_ap, n, name):
                t = rowp.tile([1, n], F32R, name=nm(name), tag="row")
                nc.sync.dma_start(
                    out=t[:],
                    in_=src_1d_ap.rearrange("(a f) -> a f", a=1).bitcast(F32R))
                return t

            def drain(units, k=None):
                n = len(units) if k is None else min(k, len(units))
                for _ in range(n):
                    units.pop(0)()

            # ---- K projection units: out kt[(n0+m0)//128][:, f0:f0+TH]
            def make_k_units(inT2d, inb1d, src_tiles, kt, wpool, upool,
                             ubufs):
                units = []
                bk_col = load_bias_col(inb1d[D:2 * D], ND, "bk")
                kst = {}
                for n0 in (0, 512):
                    def lk(n0=n0):
                        wts = []
                        for k in range(ND):
                            wt = wpool.tile([128, 512], BF16, name=nm("wk"),
                                            tag="w")
                            nc.sync.dma_start(
                                out=wt[:],
                                in_=inT2d[k * 128:(k + 1) * 128,
                                          D + n0:D + n0 + 512])
                            wts.append(wt)
                        kst[n0] = wts
                    units.append(lk)
                    for m0 in (0, 128, 256, 384):
                        for f0 in (0, TH):
                            def mmk(n0=n0, m0=m0, f0=f0):
                                wts = kst[n0]
                                ps = upool.tile([128, TH], F32, name=nm("up"),
                                                tag="up", bufs=ubufs)
                                for k in range(ND):
                                    nc.tensor.matmul(
                                        ps[:], wts[k][:, m0:m0 + 128],
                                        src_tiles[k][:, f0:f0 + TH],
                                        start=(k == 0), stop=(k == ND - 1))
                                ti = (n0 + m0) // 128
                                nc.vector.tensor_scalar_add(
                                    kt[ti][:, f0:f0 + TH], ps,
                                    bk_col[:, ti:ti + 1])
                            units.append(mmk)
                return units

            # ---- V projection units (V_aug layout [128, H*65])
            def make_v_units(inT2d, inb1d, src_tiles, va, wpool, upool,
                             ubufs):
                units = []
                bv_row = None if skip_bv else load_row(
                    inb1d[2 * D:3 * D], D, "bv")
                vst = {}
                for c0 in (0, 512):
                    def lv(c0=c0):
                        wts = []
                        for k in range(ND):
                            wt = wpool.tile([128, 512], BF16, name=nm("wv"),
                                            tag="w")
                            nc.sync.dma_start(
                                out=wt[:],
                                in_=inT2d[k * 128:(k + 1) * 128,
                                          2 * D + c0:2 * D + c0 + 512])
                            wts.append(wt)
                        vst[c0] = wts
                    units.append(lv)
                    for t in range(NKT):
                        def mmv(c0=c0, t=t):
                            wts = vst[c0]
                            ps = upool.tile([128, 512], F32, name=nm("uv"),
                                            tag="up", bufs=ubufs)
                            for k in range(ND):
                                nc.tensor.matmul(
                                    ps[:],
                                    src_tiles[k][:, t * 128:(t + 1) * 128],
                                    wts[k][:],
                                    start=(k == 0),
                                    stop=(bv_row is None and k == ND - 1))
                            if bv_row is not None:
                                nc.tensor.matmul(ps[:], ones_r128[:, :128],
                                                 bv_row[:, c0:c0 + 512],
                                                 start=False, stop=True)
                            nc.vector.tensor_copy(
                                va[t][:].rearrange("p (h e) -> p h e", e=65)
                                [:, c0 // 64:c0 // 64 + 8, 0:64],
                                ps[:].rearrange("p (h e) -> p h e", e=64))
                        units.append(mmv)
                return units

            def proj_gemm(wT2d, rhs_tiles, nout, wpool, evict, fdim=TH,
                          pre_wts=None):
                """out^T[nout, fdim] = W @ rhs. evict(n0, psum) per 128."""
                nk = len(rhs_tiles)
                ctx = tc.tile_pool(name=nm("gps"), bufs=4, space="PSUM")
                ppool = ctx.__enter__()
                for n0 in range(0, nout, 512):
                    w = min(512, nout - n0)
                    if pre_wts is not None and n0 in pre_wts:
                        wts = pre_wts[n0]
                    else:
                        wts = []
                        for k in range(nk):
                            wt = wpool.tile([128, w], BF16, name=nm("w"),
                                            tag="w")
                            nc.sync.dma_start(
                                out=wt[:],
                                in_=wT2d[k * 128:(k + 1) * 128, n0:n0 + w])
                            wts.append(wt)
                    for m0 in range(0, w, 128):
                        ps = ppool.tile([128, fdim], F32, name=nm("pg"),
                                        tag="pg", bufs=4)
                        for k in range(nk):
                            nc.tensor.matmul(
                                ps[:], wts[k][:, m0:m0 + 128],
                                rhs_tiles[k][:, :fdim],
                                start=(k == 0), stop=(k == nk - 1))
                        evict(n0 + m0, ps)
                ctx.__exit__(None, None, None)

            def prefetch_weights(wT2d, nout, wpool):
                pre = {}
                for n0 in range(0, nout, 512):
                    w = min(512, nout - n0)
                    wts = []
                    for k in range(ND):
                        wt = wpool.tile([128, w], BF16, name=nm("w"),
                                        tag="w")
                        nc.sync.dma_start(
                            out=wt[:],
                            in_=wT2d[k * 128:(k + 1) * 128, n0:n0 + w])
                        wts.append(wt)
                    pre[n0] = wts
                return pre

            # ---- LN stats emitted at producer evictions
            stats_box = [None]
            next_ag = [None]

            def stats2(stps):
                mu_ps = stps.tile([1, TH], F32, name=nm("mups"),
                                  tag="mups", bufs=1)
                s2_ps = stps.tile([1, TH], F32, name=nm("s2ps"),
                                  tag="s2ps", bufs=1)
                stats_box[0] = (mu_ps, s2_ps)
                return mu_ps, s2_ps

            def ev_stats(t, mu_ps, s2_ps):
                nc.tensor.matmul(mu_ps[:], ones_col[:], x_cur[t][:],
                                 start=(t == 0), stop=(t == ND - 1))
                sq = tmpp.tile([128, TH], F32R, name=nm("sq"), tag="tmp")
                nc.vector.tensor_tensor(sq[:], x_cur[t][:], x_cur[t][:],
                                        AOP.mult)
                nc.tensor.matmul(s2_ps[:], ones_col[:], sq[:],
                                 start=(t == 0), stop=(t == ND - 1))

            def ln_tail(mu_ps, s2_ps, w_col, w_row, stage_to=None):
                """Post-norm LN tail; stats already accumulated.
                stage_to: optional DRAM AG-staging tile — each xb16 tile is
                DMA'd there right after its cast."""
                with tc.tile_pool(name=nm("lnps"), bufs=3, space="PSUM") as lps:
                    muex = statp.tile([1, 2 * TH], F32, name=nm("muex"),
                                      tag="st2", bufs=1)
                    mu = muex[0:1, 0:TH]
                    ex2 = muex[0:1, TH:2 * TH]
                    nc.vector.tensor_scalar_mul(mu, mu_ps[:], 1.0 / D)
                    nc.vector.tensor_scalar_mul(ex2, s2_ps[:], 1.0 / D)
                    var = statp.tile([1, TH], F32, name=nm("var"), tag="st")
                    nc.vector.tensor_tensor(var[:], mu, mu, AOP.mult)
                    nc.vector.tensor_tensor(var[:], ex2, var[:],
                                            AOP.subtract)
                    sd = statp.tile([1, TH], F32R, name=nm("sd"), tag="st")
                    nc.scalar.activation(sd[:], var[:], AF.Sqrt,
                                         bias=eps_tile[:])
                    sdb_ps = lps.tile([128, TH], F32, name=nm("sdb"), bufs=1)
                    nc.tensor.matmul(sdb_ps[:], ones_r128[:], sd[:],
                                     start=True, stop=True)
                    rs_b = tmpp.tile([128, TH], F32, name=nm("rsb"),
                                     tag="rb", bufs=2)
                    nc.vector.reciprocal_approx_fast(out=rs_b[:],
                                                     in_=sdb_ps[:])
                    nc.vector.tensor_tensor(lnrhs[0:1, :], mu, rs_b[0:1, :],
                                            AOP.mult)
                    for t in range(ND):
                        aux = lps.tile([128, TH], F32, name=nm("aux"),
                                       tag="lnaux", bufs=2)
                        nc.tensor.matmul(aux[:],
                                         w_row[:, t * 128:(t + 1) * 128],
                                         lnrhs[:], start=True, stop=True)
                        t1 = tmpp.tile([128, TH], F32R, name=nm("t1"),
                                       tag="tmp")
                        nc.vector.tensor_tensor(t1[:], x_cur[t][:],
                                                rs_b[:], AOP.mult)
                        nc.vector.scalar_tensor_tensor(
                            x_cur[t][:], t1[:], w_col[:, t:t + 1], aux[:],
                            AOP.mult, AOP.subtract)
                        nc.vector.tensor_copy(xb16[t][:], x_cur[t][:])
                        if stage_to is not None:
                            nc.sync.dma_start(
                                out=stage_to[t * 128:(t + 1) * 128, :],
                                in_=xb16[t][:])

            def ln_wcol_wrow(idx, l):
                lwb = rowp.tile([2, D], F32R, name=nm(f"ln{idx}wb"),
                                tag="row")
                nc.sync.dma_start(
                    out=lwb[0:1, :],
                    in_=lnw_d[idx].ap()[l].rearrange(
                        "(a f) -> a f", a=1).bitcast(F32R))
                nc.sync.dma_start(
                    out=lwb[1:2, :],
                    in_=lnb_d[idx].ap()[l].rearrange(
                        "(a f) -> a f", a=1).bitcast(F32R))
                lwc = load_bias_col(lnw_d[idx].ap()[l], ND, f"ln{idx}wc")
                return lwc, lwb

            # ---- attention: heads in duos, fused strided exp
            def attention(pp, q_tiles, kt_tiles, vaug_tiles, restrict,
                          kval, filler):
                at = [pp.tile([128, TH], BF16, name=nm("at"), tag="attnT",
                              bufs=8) for _ in range(ND)]
                with (
                    tc.tile_pool(name=nm("aps"), bufs=2, space="PSUM") as sps,
                    tc.tile_pool(name=nm("ops"), bufs=2, space="PSUM") as ops,
                    tc.tile_pool(name=nm("bps"), bufs=1, space="PSUM") as bps,
                ):
                    for h0 in range(0, H, 2):
                        ti = h0 // 2
                        o0 = ops.tile([65, TH], F32, name=nm("o0"),
                                      tag="po", bufs=2)
                        o1 = ops.tile([65, TH], F32, name=nm("o1"),
                                      tag="po", bufs=2)
                        for t in range(NKT):
                            rq = RQ0 if (restrict and t >= 5) else 0
                            s = sps.tile([128, 1024], F32, name=nm("s"),
                                         tag="ps", bufs=2)
                            nc.tensor.matmul(
                                s[:, rq:TH],
                                kt_tiles[ti][0:64, t * 128:(t + 1) * 128],
                                q_tiles[ti][0:64, rq:TH],
                                start=True, stop=True)
                            nc.tensor.matmul(
                                s[:, 512 + rq:512 + TH],
                                kt_tiles[ti][64:128, t * 128:(t + 1) * 128],
                                q_tiles[ti][64:128, rq:TH],
                                start=True, stop=True)
                            if filler:
                                drain(filler, 1)
                            sv = s[:].rearrange("p (a q) -> p a q",
                                                q=512)[:, :, rq:TH]
                            pb = tmpp.tile([128, 2 * TH], BF16, name=nm("pb"),
                                           tag="pexp", bufs=3)
                            pv = pb[:].rearrange("p (a q) -> p a q",
                                                 q=TH)[:, :, rq:TH]
                            if restrict and t >= 5:
                                mt = mask5 if t == 5 else mask6
                                nc.vector.tensor_tensor(
                                    s[:, rq:TH], s[:, rq:TH], mt[:], AOP.add)
                                nc.vector.tensor_tensor(
                                    s[:, 512 + rq:512 + TH],
                                    s[:, 512 + rq:512 + TH], mt[:], AOP.add)
                            if kval is not None and t == NKT - 1:
                                nc.scalar.activation(pv, sv, AF.Exp,
                                                     bias=kval[:])
                            else:
                                nc.scalar.activation(pv, sv, AF.Exp)
                            vaw = vaug_tiles[t][:].rearrange(
                                "p (h e) -> p h e", e=65)
                            nc.tensor.matmul(
                                o0[:, rq:TH], vaw[:, h0, :],
                                pb[:, rq:TH],
                                start=(t == 0), stop=(t == NKT - 1))
                            nc.tensor.matmul(
                                o1[:, rq:TH], vaw[:, h0 + 1, :],
                                pb[:, TH + rq:2 * TH],
                                start=(t == 0), stop=(t == NKT - 1))
                        # softmax denominators for the duo
                        d0 = statp.tile([1, TH], F32R, name=nm("d0"),
                                        tag="rd", bufs=2)
                        d1 = statp.tile([1, TH], F32R, name=nm("d1"),
                                        tag="rd", bufs=2)
                        nc.vector.tensor_copy(d0[:], o0[64:65, :])
                        nc.vector.tensor_copy(d1[:], o1[64:65, :])
                        r_ps = bps.tile([128, TH], F32, name=nm("rps"),
                                        tag="pb1", bufs=1)
                        nc.tensor.matmul(r_ps[:], k2sel_a[:], d0[:],
                                         start=True, stop=False)
                        nc.tensor.matmul(r_ps[:], k2sel_b[:], d1[:],
                                         start=False, stop=True)
                        rb = tmpp.tile([128, TH], F32, name=nm("rb"),
                                       tag="rb", bufs=2)
                        nc.vector.reciprocal_approx_fast(out=rb[:],
                                                         in_=r_ps[:])
                        nc.vector.tensor_tensor(at[ti][0:64, :],
                                                o0[0:64, :], rb[0:64, :],
                                                AOP.mult)
                        nc.vector.tensor_tensor(at[ti][64:128, :],
                                                o1[0:64, :], rb[64:128, :],
                                                AOP.mult)
                return at

            def qproj(pp, inT2d, inb1d, pre):
                q_t = [pp.tile([128, TH], BF16, name=nm("q"), tag="q",
                               bufs=8) for _ in range(ND)]
                bq_col = load_bias_col(inb1d[0:D], ND, "bq")

                def ev_q(n0, ps):
                    nc.vector.tensor_scalar_add(
                        q_t[n0 // 128][:], ps,
                        bq_col[:, n0 // 128:n0 // 128 + 1])
                proj_gemm(inT2d[:, 0:D], xb16, D, wA, ev_q, pre_wts=pre)
                return q_t

            def out_proj(wT2d, b1d, at, mu_ps, s2_ps):
                bo_col = load_bias_col(b1d, ND, "bo")

                def ev_o(n0, ps):
                    t = n0 // 128
                    nc.vector.scalar_tensor_tensor(
                        x_cur[t][:], ps, bo_col[:, t:t + 1], x_cur[t][:],
                        AOP.add, AOP.add)
                    ev_stats(t, mu_ps, s2_ps)
                proj_gemm(wT2d, at, D, wA, ev_o)

            # ================================================= layer loop
            ca_k_units = None      # K units for layer l (made in l-1)
            ca_kt = None           # CA K^T tiles for layer l
            for l in range(n_layers):
                # CA V tiles + units for this layer (drained AG gap + SA win)
                va_ca = [cakvp.tile([128, H * 65], BF16, name=nm("vca"),
                                    tag="vca", bufs=NKT) for _ in range(NKT)]
                for t in range(NKT):
                    nc.sync.dma_start(
                        out=va_ca[t][:].rearrange("p (h e) -> p h e", e=65)
                        [:, :, 64:65],
                        in_=vones_d.ap())
                ca_v_units = make_v_units(ca_inT_d.ap()[l], ca_inb_d.ap()[l],
                                          memt, va_ca, wU, upsp, 1)
                if ca_k_units is None:
                    # layer 0: K units made+drained here
                    ca_kt = [cakvp.tile([128, TPAD], BF16, name=nm("kca"),
                                        tag="kca", bufs=2 * ND)
                             for _ in range(ND)]
                    ca_k_units = make_k_units(
                        ca_inT_d.ap()[l], ca_inb_d.ap()[l], memt, ca_kt,
                        wU, upsp, 1)
                    drain(ca_k_units)
                kt_ca, va_ca_l = ca_kt, va_ca

                # ---- AllGather of x halves (staged during prev LN3 tail)
                if l == 0:
                    ag_in = dramp.tile([D, TH], BF16, name=nm("agin"),
                                       tag="agi")
                    for t in range(ND):
                        nc.sync.dma_start(
                            out=ag_in[t * 128:(t + 1) * 128, :],
                            in_=xb16[t][:])
                else:
                    ag_in = next_ag[0]
                ag_out = dramp.tile([2 * D, TH], BF16, name=nm("agout"),
                                    tag="ago")
                nc.gpsimd.collective_compute(
                    "AllGather", mybir.AluOpType.bypass,
                    replica_groups=[[0, 1], [2, 3], [4, 5], [6, 7]],
                    ins=[ag_in[:].opt()], outs=[ag_out[:].opt()])

                with tc.tile_pool(name=nm("att_sb"), bufs=2) as pp:
                    drain(ca_v_units, 6)          # AG-gap filler
                    pre_q = prefetch_weights(sa_inT_d.ap()[l][:, 0:D], D, wA)
                    q_sa = qproj(pp, sa_inT_d.ap()[l], sa_inb_d.ap()[l],
                                 pre_q)
                    drain(ca_v_units, 2)

                    # SA K/V from the gathered full sequence
                    xfull = [pp.tile([128, TPAD], BF16, name=nm("xf"),
                                     tag="xfull", bufs=8) for _ in range(ND)]
                    for t in range(ND):
                        nc.sync.dma_start(
                            out=xfull[t][:, 0:TH],
                            in_=ag_out[t * 128:(t + 1) * 128, :])
                        nc.sync.dma_start(
                            out=xfull[t][:, TH:TPAD],
                            in_=ag_out[D + t * 128:D + (t + 1) * 128, :])
                    kt_sa = [pp.tile([128, TPAD], BF16, name=nm("ksa"),
                                     tag="ksa", bufs=8) for _ in range(ND)]
                    va_sa = [pp.tile([128, H * 65], BF16, name=nm("vsa"),
                                     tag="vsa", bufs=NKT) for _ in range(NKT)]
                    for t in range(NKT):
                        nc.sync.dma_start(
                            out=va_sa[t][:].rearrange("p (h e) -> p h e",
                                                      e=65)[:, :, 64:65],
                            in_=vones_d.ap())
                    with tc.tile_pool(name=nm("sakv"), bufs=4,
                                      space="PSUM") as sap:
                        sk = make_k_units(sa_inT_d.ap()[l],
                                          sa_inb_d.ap()[l], xfull, kt_sa,
                                          wA, sap, 4)
                        drain(sk)
                        sv = make_v_units(sa_inT_d.ap()[l],
                                          sa_inb_d.ap()[l], xfull, va_sa,
                                          wA, sap, 4)
                        drain(sv)

                    at = attention(pp, q_sa, kt_sa, va_sa, True, None,
                                   ca_v_units)
                    drain(ca_v_units)
                    lwc1, lwb1 = ln_wcol_wrow(0, l)
                    with tc.tile_pool(name=nm("st1"), bufs=2,
                                      space="PSUM") as stps:
                        out_proj(sa_outT_d.ap()[l], sa_outb_d.ap()[l], at,
                                 *stats2(stps))
                        ln_tail(*stats_box[0], lwc1, lwb1)

                    q_ca = qproj(pp, ca_inT_d.ap()[l], ca_inb_d.ap()[l],
                                 None)

                    # K units for layer l+1 fill this CA window
                    next_filler = None
                    if l + 1 < n_layers:
                        ca_kt = [cakvp.tile([128, TPAD], BF16,
                                            name=nm("kca"), tag="kca",
                                            bufs=2 * ND) for _ in range(ND)]
                        ca_k_units = make_k_units(
                            ca_inT_d.ap()[l + 1], ca_inb_d.ap()[l + 1],
                            memt, ca_kt, wU, upsp, 1)
                        next_filler = ca_k_units
                    at = attention(pp, q_ca, kt_ca, va_ca_l, False, kval6,
                                   next_filler)
                    if next_filler:
                        drain(next_filler)
                        ca_k_units = []
                    lwc2, lwb2 = ln_wcol_wrow(1, l)
                    with tc.tile_pool(name=nm("st2"), bufs=2,
                                      space="PSUM") as stps:
                        out_proj(ca_outT_d.ap()[l], ca_outb_d.ap()[l], at,
                                 *stats2(stps))
                        ln_tail(*stats_box[0], lwc2, lwb2)

                # ================= FFN =================
                with tc.tile_pool(name=nm("ff_sb"), bufs=2) as pp:
                    ht = [pp.tile([128, TH], BF16, name=nm("h"), tag="h",
                                  bufs=FF // 128) for _ in range(FF // 128)]
                    b1_col = load_bias_col(ff1b_d.ap()[l], FF // 128, "b1")

                    def ev_h(n0, ps):
                        t = n0 // 128
                        nc.scalar.activation(
                            ht[t][:], ps, AF.Relu, bias=b1_col[:, t:t + 1])
                    proj_gemm(ff1T_d.ap()[l], xb16, FF, wB, ev_h)

                    b2_col = load_bias_col(ff2b_d.ap()[l], ND, "b2")
                    with tc.tile_pool(name=nm("st3"), bufs=2,
                                      space="PSUM") as stps:
                        mu3, s23 = stats2(stps)

                        def ev_f(n0, ps):
                            t = n0 // 128
                            nc.vector.scalar_tensor_tensor(
                                x_cur[t][:], ps, b2_col[:, t:t + 1],
                                x_cur[t][:], AOP.add, AOP.add)
                            ev_stats(t, mu3, s23)
                        proj_gemm(ff2T_d.ap()[l], ht, D, wB, ev_f)

                        lwc3, lwb3 = ln_wcol_wrow(2, l)
                        if l + 1 < n_layers:
                            ag_in2 = dramp.tile([D, TH], BF16,
                                                name=nm("agin"), tag="agi")
                            ln_tail(mu3, s23, lwc3, lwb3, stage_to=ag_in2)
                            next_ag[0] = ag_in2
                        else:
                            ln_tail(mu3, s23, lwc3, lwb3)

            # ---- final projection
            ob_col = load_bias_col(outb_d.ap(), VOCAB // 128, "ob")

            def ev_y(n0, ps):
                y = tmpp.tile([128, TH], F32, name=nm("y"), tag="tmp")
                nc.vector.tensor_scalar_add(
                    y[:], ps, ob_col[:, n0 // 128:n0 // 128 + 1])
                nc.sync.dma_start(out=yT_d.ap()[n0:n0 + 128, :], in_=y[:])
            proj_gemm(outT_d.ap(), xb16, VOCAB, wA, ev_y)

    nc.compile()
    return nc


# ---------------------------------------------------------------- wrapper

def prep_in_maps(inputs):
    f32 = lambda a: np.ascontiguousarray(np.asarray(a, dtype=np.float32))
    embed = host_embed(
        np.asarray(inputs["text"]), np.asarray(inputs["audio"]),
        np.asarray(inputs["enrolled_audio"]),
        np.asarray(inputs["text_len_batch"]),
        np.asarray(inputs["audio_len_batch"]),
        f32(inputs["text_emb"]), f32(inputs["audio_emb"]))
    embT = np.ascontiguousarray(embed.transpose(0, 2, 1))   # [B, D, TPAD]

    bf = lambda a: np.ascontiguousarray(a.astype(np.float16))
    tr = lambda a: np.ascontiguousarray(
        np.asarray(a, dtype=np.float32).transpose(0, 2, 1))
    sa_inT = tr(inputs["sa_in_w"])      # [L, D, 3D]
    ca_inT = tr(inputs["ca_in_w"])
    sa_inT[:, :, :D] *= 0.125           # fold 1/sqrt(hd) into Q
    ca_inT[:, :, :D] *= 0.125
    sa_inb = f32(inputs["sa_in_b"]).copy()
    ca_inb = f32(inputs["ca_in_b"]).copy()
    sa_inb[:, :D] *= 0.125
    ca_inb[:, :D] *= 0.125

    shared = dict(
        kval6=host_kvalid6(),
        ones_col=np.ones((128, 1), np.float32),
        ones_r128=np.ones((1, 128), np.float32),
        vones=np.ones((128, H), np.float16),
        neg_r448=np.full((1, 448), -1.0, np.float32),
        k2sel=np.concatenate([
            np.concatenate([np.ones((1, 64)), np.zeros((1, 64))], 1),
            np.concatenate([np.zeros((1, 64)), np.ones((1, 64))], 1),
        ]).astype(np.float32),
        sa_inT=bf(sa_inT), sa_outT=bf(tr(inputs["sa_out_w"])),
        ca_inT=bf(ca_inT), ca_outT=bf(tr(inputs["ca_out_w"])),
        ff1T=bf(tr(inputs["ff1_w"])), ff2T=bf(tr(inputs["ff2_w"])),
        outT=bf(np.ascontiguousarray(f32(inputs["out_w"]).T)),
        sa_inb=sa_inb, sa_outb=f32(inputs["sa_out_b"]),
        ca_inb=ca_inb, ca_outb=f32(inputs["ca_out_b"]),
        ff1b=f32(inputs["ff1_b"]), ff2b=f32(inputs["ff2_b"]),
        outb=f32(inputs["out_b"]),
        ln1w=f32(inputs["ln1_w"]), ln1b=f32(inputs["ln1_b"]),
        ln2w=f32(inputs["ln2_w"]), ln2b=f32(inputs["ln2_b"]),
        ln3w=f32(inputs["ln3_w"]), ln3b=f32(inputs["ln3_b"]),
    )
    in_maps = []
    for c in range(8):
        bb, hh = c // 2, c % 2
        m = dict(shared)
        m["xT0"] = np.ascontiguousarray(embT[bb][:, hh * TH:(hh + 1) * TH])
        m["memT"] = bf(embT[bb])
        m["maskT"] = host_masks(hh)
        in_maps.append(m)
    return in_maps


_NC_CACHE = {}


def run(inputs, n_layers=L, trace=False):
    skip_bv = (not np.any(np.asarray(inputs["sa_in_b"])[:, 2 * D:])
               and not np.any(np.asarray(inputs["ca_in_b"])[:, 2 * D:]))
    key = (n_layers, skip_bv)
    if key not in _NC_CACHE:
        _NC_CACHE[key] = build_kernel(n_layers, skip_bv)
    nc = _NC_CACHE[key]
    in_maps = prep_in_maps(inputs)
    res = bass_utils.run_bass_kernel_spmd(
        nc, in_maps, core_ids=list(range(8)), trace=trace)
    out = np.zeros((B, SEQ, VOCAB), dtype=np.float32)
    for c in range(8):
        bb, hh = c // 2, c % 2
        cols = TH if hh == 0 else SEQ - TH
        out[bb, hh * TH:hh * TH + cols, :] = \
            res.results[c]["yT"][:, :cols].T
    return out, res


def kernel(**inputs):
    out, _ = run(inputs)
    return out
